# revision 8
# baseline (speedup 1.0000x reference)
"""Trainium2 kernel for nn_DifferentiableBiquad.

Cascade of 4 biquad IIR filters over (B=32, L=524288), f32.

Algorithm: the pole radii are sigmoid(logit)*0.999 (actual inputs give
r_max ~ 0.61), so the cascade impulse response decays below f32
resolution within ~64 lags. The IIR is therefore computed exactly (to
f32 precision) as a truncated FIR, expressed as block-Toeplitz matmuls
on the TensorEngine:

  - x (per batch row) is split into 128-sample blocks. SBUF holds
    xt[m, b] = x[b*128 + m] (obtained with a PE transpose of the
    naturally-loaded tile).
  - For each window of 128 blocks, with stationary = 128 columns of xt:
      psum[b, n]    = sum_m xt[m, wb+b]   * H0T[m, n]  (lags 0..127)
      psum[b, :NC1] += sum_m xt[m, wb+b-1] * H1T[m, n] (lags 128+n-m)
    giving y in natural layout [block, within-block].
  - DVE evicts PSUM into a staging buffer, DMA'd out contiguously.

Batch dim (32) is sharded over 8 NeuronCores (4 rows each); rows are
independent (zero initial state == zero history blocks).
"""
import math
import os

import numpy as np

NUM_FILTERS = 4
MAX_RADIUS = 0.999
B, L = 32, 524288
N_CORES = 8
ROWS_PER_CORE = B // N_CORES
NBLK = 128  # block size == SBUF partitions


# ---------------------------------------------------------------- host math
def _coeffs_f32(log_radius, raw_angle):
    lr = np.asarray(log_radius, np.float32)
    ra = np.asarray(raw_angle, np.float32)
    radius = (np.float32(1.0) / (np.float32(1.0) + np.exp(-lr, dtype=np.float32))) * np.float32(MAX_RADIUS)
    angle = (np.float32(1.0) / (np.float32(1.0) + np.exp(-ra, dtype=np.float32))) * np.float32(math.pi)
    a1 = np.float32(-2.0) * radius * np.cos(angle, dtype=np.float32)
    a2 = radius * radius
    return a1.astype(np.float32), a2.astype(np.float32)


def _impulse_response(a1, a2, b0, b1, b2, T=256):
    h = np.zeros(T, np.float64)
    h[0] = 1.0
    for f in range(NUM_FILTERS):
        s1 = s2 = 0.0
        out = np.zeros(T, np.float64)
        for n in range(T):
            xn = h[n]
            yn = float(b0[f]) * xn + s1
            s1 = float(b1[f]) * xn - float(a1[f]) * yn + s2
            s2 = float(b2[f]) * xn - float(a2[f]) * yn
            out[n] = yn
        h = out
    return h


def _build_tap_matrices(inputs):
    a1, a2 = _coeffs_f32(inputs["log_radius"], inputs["raw_angle"])
    h = _impulse_response(
        a1, a2,
        np.asarray(inputs["b0"], np.float64),
        np.asarray(inputs["b1"], np.float64),
        np.asarray(inputs["b2"], np.float64),
    )
    hmax = np.abs(h).max()
    tap_max = int(np.max(np.nonzero(np.abs(h) > 1e-10 * hmax)))
    assert tap_max <= 127, (
        f"impulse response too long for single-shift kernel (tap_max={tap_max})"
    )
    NC1 = max(1, min(128, tap_max))
    n_idx = np.arange(NBLK)
    m_idx = np.arange(NBLK)
    lag0 = n_idx[None, :] - m_idx[:, None]          # [m, n]
    H0T = np.where((lag0 >= 0) & (lag0 <= tap_max), h[np.clip(lag0, 0, 255)], 0.0)
    lag1 = 128 + n_idx[None, :NC1] - m_idx[:, None]  # [m, n]
    H1T = np.where((lag1 >= 1) & (lag1 <= tap_max), h[np.clip(lag1, 0, 255)], 0.0)
    return H0T.astype(np.float32), H1T.astype(np.float32)


# ---------------------------------------------------------------- program
_PROGRAM_CACHE = {}


def build_program(n_rows, length, NC1):
    import concourse.bass as bass
    import concourse.mybir as mybir
    from concourse import bacc
    from concourse.tile import TileContext

    f32 = mybir.dt.float32
    nblocks = length // NBLK
    nwin = nblocks // NBLK
    gsz = min(4, nwin)                 # windows per psum-transpose group
    ngroups = nwin // gsz
    assert nwin % gsz == 0 and nblocks % NBLK == 0 and length % NBLK == 0

    use_f32r = bool(int(os.environ.get("F32R", "0")))
    mmdt = mybir.dt.float32r if use_f32r else f32

    nc = bacc.Bacc("TRN2", target_bir_lowering=False, debug=False,
                   num_devices=N_CORES)
    xin = nc.dram_tensor("xin", [n_rows, length], f32, kind="ExternalInput")
    h0t = nc.dram_tensor("h0t", [NBLK, NBLK], mmdt, kind="ExternalInput")
    h1t = nc.dram_tensor("h1t", [NBLK, NC1], mmdt, kind="ExternalInput")
    ident = nc.dram_tensor("ident", [NBLK, NBLK], f32, kind="ExternalInput")
    yout = nc.dram_tensor("yout", [n_rows, length], f32, kind="ExternalOutput")

    with TileContext(nc) as tc:
        with (
            tc.tile_pool(name="const", bufs=1) as cpool,
            tc.tile_pool(name="vrow", bufs=4) as vpool,
            tc.tile_pool(name="xt", bufs=3) as xtpool,
            tc.tile_pool(name="stage", bufs=4) as spool,
            tc.tile_pool(name="pt", bufs=2, space="PSUM") as ptpool,
            tc.tile_pool(name="py", bufs=4, space="PSUM") as pypool,
        ):
            h0_sb = cpool.tile([NBLK, NBLK], mmdt, tag="h0")
            nc.sync.dma_start(out=h0_sb[:], in_=h0t.ap())
            h1_sb = cpool.tile([NBLK, NC1], mmdt, tag="h1")
            nc.sync.dma_start(out=h1_sb[:], in_=h1t.ap())
            id_sb = cpool.tile([NBLK, NBLK], f32, tag="id")
            nc.sync.dma_start(out=id_sb[:], in_=ident.ap())
            zcol = cpool.tile([NBLK, 1], f32, tag="zc")
            nc.gpsimd.memset(zcol[:], 0.0)

            # Input groups: per group, partition q holds gsz*128 contiguous
            # samples (one 2KB DMA run); transpose j recovers every-gsz-th
            # 128-block; a strided DVE evict restores consecutive-block xt.
            xin_v = xin.ap().rearrange(
                "r (g q j m) -> r g q j m", q=NBLK, j=gsz, m=NBLK
            )
            yout_v = yout.ap().rearrange(
                "r (g k p n) -> r g p k n", k=gsz, p=NBLK, n=NBLK
            )
            for r in range(n_rows):
                prev_xt = None
                for g in range(ngroups):
                    v = vpool.tile([NBLK, gsz, NBLK], f32, tag="v")
                    nc.sync.dma_start(out=v[:], in_=xin_v[r, g])
                    pt = ptpool.tile([NBLK, gsz * NBLK], f32, tag="pt")
                    for j in range(gsz):
                        nc.tensor.transpose(
                            pt[:, j * NBLK:(j + 1) * NBLK], v[:, j, :], id_sb[:]
                        )
                    xt = xtpool.tile([NBLK, gsz * NBLK + 1], mmdt, tag="xt")
                    nc.vector.tensor_copy(
                        out=xt[:, 1:].rearrange("p (q j) -> p j q", j=gsz),
                        in_=pt[:].rearrange("p (j q) -> p j q", q=NBLK),
                    )
                    if g == 0:
                        nc.vector.tensor_copy(out=xt[:, 0:1], in_=zcol[:])
                    else:
                        nc.vector.tensor_copy(
                            out=xt[:, 0:1], in_=prev_xt[:, gsz * NBLK:gsz * NBLK + 1]
                        )
                    prev_xt = xt
                    stage = spool.tile([NBLK, gsz, NBLK], f32, tag="stage")
                    py = pypool.tile([NBLK, gsz * NBLK], f32, tag="py")
                    for k in range(gsz):
                        nc.tensor.matmul(
                            py[:, k * NBLK:(k + 1) * NBLK],
                            xt[:, 1 + k * NBLK:1 + (k + 1) * NBLK],
                            h0_sb[:],
                            start=True, stop=False, skip_group_check=True,
                        )
                        nc.tensor.matmul(
                            py[:, k * NBLK:k * NBLK + NC1],
                            xt[:, k * NBLK:(k + 1) * NBLK],
                            h1_sb[:],
                            start=False, stop=True, skip_group_check=True,
                        )
                    nc.vector.tensor_copy(
                        out=stage[:],
                        in_=py[:].rearrange("p (k n) -> p k n", n=NBLK),
                    )
                    nc.scalar.dma_start(out=yout_v[r, g], in_=stage[:])
    nc.compile()
    return nc


def _get_program(n_rows, length, NC1):
    key = (n_rows, length, NC1)
    if key not in _PROGRAM_CACHE:
        _PROGRAM_CACHE[key] = build_program(*key)
    return _PROGRAM_CACHE[key]


# ---------------------------------------------------------------- entry
def _run(inputs, trace=False):
    from concourse.bass_utils import run_bass_kernel_spmd

    x = np.ascontiguousarray(np.asarray(inputs["x"], np.float32))
    assert x.shape == (B, L)
    H0T, H1T = _build_tap_matrices(inputs)
    NC1 = H1T.shape[1]
    I = np.eye(NBLK, dtype=np.float32)

    nc = _get_program(ROWS_PER_CORE, L, NC1)
    xs = x.reshape(N_CORES, ROWS_PER_CORE, L)
    in_maps = [
        {"xin": xs[c], "h0t": H0T, "h1t": H1T, "ident": I}
        for c in range(N_CORES)
    ]
    res = run_bass_kernel_spmd(nc, in_maps, core_ids=list(range(N_CORES)),
                               trace=trace)
    y = np.concatenate(
        [np.asarray(res.results[c]["yout"], np.float32) for c in range(N_CORES)],
        axis=0,
    ).reshape(B, L)
    return y, res


def kernel(x, log_radius, raw_angle, b0, b1, b2):
    y, _ = _run(dict(x=x, log_radius=log_radius, raw_angle=raw_angle,
                     b0=b0, b1=b1, b2=b2))
    return y


# revision 9
# speedup vs baseline: 1.1468x; 1.1468x over previous
"""Trainium2 kernel for nn_DifferentiableBiquad.

Cascade of 4 biquad IIR filters over (B=32, L=524288), f32.

Algorithm: the pole radii are sigmoid(logit)*0.999 (actual inputs give
r_max ~ 0.61), so the cascade impulse response decays below f32
resolution within ~64 lags. The IIR is therefore computed exactly (to
f32 precision) as a truncated FIR, expressed as block-Toeplitz matmuls
on the TensorEngine:

  - x (per batch row) is split into 128-sample blocks. SBUF holds
    xt[m, b] = x[b*128 + m] (obtained with a PE transpose of the
    naturally-loaded tile).
  - For each window of 128 blocks, with stationary = 128 columns of xt:
      psum[b, n]    = sum_m xt[m, wb+b]   * H0T[m, n]  (lags 0..127)
      psum[b, :NC1] += sum_m xt[m, wb+b-1] * H1T[m, n] (lags 128+n-m)
    giving y in natural layout [block, within-block].
  - DVE evicts PSUM into a staging buffer, DMA'd out contiguously.

Batch dim (32) is sharded over 8 NeuronCores (4 rows each); rows are
independent (zero initial state == zero history blocks).
"""
import math
import os

import numpy as np

NUM_FILTERS = 4
MAX_RADIUS = 0.999
B, L = 32, 524288
N_CORES = 8
ROWS_PER_CORE = B // N_CORES
NBLK = 128  # block size == SBUF partitions


# ---------------------------------------------------------------- host math
def _coeffs_f32(log_radius, raw_angle):
    lr = np.asarray(log_radius, np.float32)
    ra = np.asarray(raw_angle, np.float32)
    radius = (np.float32(1.0) / (np.float32(1.0) + np.exp(-lr, dtype=np.float32))) * np.float32(MAX_RADIUS)
    angle = (np.float32(1.0) / (np.float32(1.0) + np.exp(-ra, dtype=np.float32))) * np.float32(math.pi)
    a1 = np.float32(-2.0) * radius * np.cos(angle, dtype=np.float32)
    a2 = radius * radius
    return a1.astype(np.float32), a2.astype(np.float32)


def _impulse_response(a1, a2, b0, b1, b2, T=256):
    h = np.zeros(T, np.float64)
    h[0] = 1.0
    for f in range(NUM_FILTERS):
        s1 = s2 = 0.0
        out = np.zeros(T, np.float64)
        for n in range(T):
            xn = h[n]
            yn = float(b0[f]) * xn + s1
            s1 = float(b1[f]) * xn - float(a1[f]) * yn + s2
            s2 = float(b2[f]) * xn - float(a2[f]) * yn
            out[n] = yn
        h = out
    return h


def _build_tap_matrices(inputs):
    a1, a2 = _coeffs_f32(inputs["log_radius"], inputs["raw_angle"])
    h = _impulse_response(
        a1, a2,
        np.asarray(inputs["b0"], np.float64),
        np.asarray(inputs["b1"], np.float64),
        np.asarray(inputs["b2"], np.float64),
    )
    hmax = np.abs(h).max()
    tap_max = int(np.max(np.nonzero(np.abs(h) > 1e-10 * hmax)))
    assert tap_max <= 127, (
        f"impulse response too long for single-shift kernel (tap_max={tap_max})"
    )
    NC1 = max(1, min(128, tap_max))
    n_idx = np.arange(NBLK)
    m_idx = np.arange(NBLK)
    lag0 = n_idx[None, :] - m_idx[:, None]          # [m, n]
    H0T = np.where((lag0 >= 0) & (lag0 <= tap_max), h[np.clip(lag0, 0, 255)], 0.0)
    lag1 = 128 + n_idx[None, :NC1] - m_idx[:, None]  # [m, n]
    H1T = np.where((lag1 >= 1) & (lag1 <= tap_max), h[np.clip(lag1, 0, 255)], 0.0)
    return H0T.astype(np.float32), H1T.astype(np.float32)


# ---------------------------------------------------------------- program
_PROGRAM_CACHE = {}


def build_program(n_rows, length, NC1):
    import concourse.bass as bass
    import concourse.mybir as mybir
    from concourse import bacc
    from concourse.tile import TileContext

    f32 = mybir.dt.float32
    nblocks = length // NBLK
    nwin = nblocks // NBLK
    gsz = min(4, nwin)                 # windows per psum-transpose group
    ngroups = nwin // gsz
    assert nwin % gsz == 0 and nblocks % NBLK == 0 and length % NBLK == 0

    use_f32r = bool(int(os.environ.get("F32R", "0")))
    mmdt = mybir.dt.float32r if use_f32r else f32

    nc = bacc.Bacc("TRN2", target_bir_lowering=False, debug=False,
                   num_devices=N_CORES)
    xin = nc.dram_tensor("xin", [n_rows, length], f32, kind="ExternalInput")
    h0t = nc.dram_tensor("h0t", [NBLK, NBLK], mmdt, kind="ExternalInput")
    h1t = nc.dram_tensor("h1t", [NBLK, NC1], mmdt, kind="ExternalInput")
    ident = nc.dram_tensor("ident", [NBLK, NBLK], f32, kind="ExternalInput")
    yout = nc.dram_tensor("yout", [n_rows, length], f32, kind="ExternalOutput")

    with TileContext(nc) as tc:
        with (
            tc.tile_pool(name="const", bufs=1) as cpool,
            tc.tile_pool(name="vrow", bufs=4) as vpool,
            tc.tile_pool(name="xt", bufs=3) as xtpool,
            tc.tile_pool(name="stage", bufs=4) as spool,
            tc.tile_pool(name="pt", bufs=3, space="PSUM") as ptpool,
            tc.tile_pool(name="py", bufs=4, space="PSUM") as pypool,
        ):
            h0_sb = cpool.tile([NBLK, NBLK], mmdt, tag="h0")
            nc.sync.dma_start(out=h0_sb[:], in_=h0t.ap())
            h1_sb = cpool.tile([NBLK, NC1], mmdt, tag="h1")
            nc.sync.dma_start(out=h1_sb[:], in_=h1t.ap())
            id_sb = cpool.tile([NBLK, NBLK], f32, tag="id")
            nc.sync.dma_start(out=id_sb[:], in_=ident.ap())
            zcol = cpool.tile([NBLK, 1], f32, tag="zc")
            nc.gpsimd.memset(zcol[:], 0.0)

            # Input groups: per group, partition q holds gsz*128 contiguous
            # samples (one 2KB DMA run); transpose j recovers every-gsz-th
            # 128-block; a strided DVE evict restores consecutive-block xt.
            xin_v = xin.ap().rearrange(
                "r (g q j m) -> r g q j m", q=NBLK, j=gsz, m=NBLK
            )
            yout_v = yout.ap().rearrange(
                "r (g k p n) -> r g p k n", k=gsz, p=NBLK, n=NBLK
            )
            for r in range(n_rows):
                prev_xt = None
                for g in range(ngroups):
                    v = vpool.tile([NBLK, gsz, NBLK], f32, tag="v")
                    nc.sync.dma_start(out=v[:], in_=xin_v[r, g])
                    pt = ptpool.tile([NBLK, gsz * NBLK], f32, tag="pt")
                    for j in range(gsz):
                        nc.tensor.transpose(
                            pt[:, j * NBLK:(j + 1) * NBLK], v[:, j, :], id_sb[:]
                        )
                    xt = xtpool.tile([NBLK, gsz * NBLK + 1], mmdt, tag="xt")
                    nc.vector.tensor_copy(
                        out=xt[:, 1:].rearrange("p (q j) -> p j q", j=gsz),
                        in_=pt[:].rearrange("p (j q) -> p j q", q=NBLK),
                    )
                    if g == 0:
                        nc.vector.tensor_copy(out=xt[:, 0:1], in_=zcol[:])
                    else:
                        nc.vector.tensor_copy(
                            out=xt[:, 0:1], in_=prev_xt[:, gsz * NBLK:gsz * NBLK + 1]
                        )
                    prev_xt = xt
                    stage = spool.tile([NBLK, gsz, NBLK], f32, tag="stage")
                    for k in range(gsz):
                        py = pypool.tile([NBLK, NBLK], f32, tag="py")
                        nc.tensor.matmul(
                            py[:],
                            xt[:, 1 + k * NBLK:1 + (k + 1) * NBLK],
                            h0_sb[:],
                            start=True, stop=False,
                        )
                        nc.tensor.matmul(
                            py[:, 0:NC1],
                            xt[:, k * NBLK:(k + 1) * NBLK],
                            h1_sb[:],
                            start=False, stop=True,
                        )
                        nc.vector.tensor_copy(out=stage[:, k, :], in_=py[:])
                    nc.scalar.dma_start(out=yout_v[r, g], in_=stage[:])
    nc.compile()
    return nc


def _get_program(n_rows, length, NC1):
    key = (n_rows, length, NC1)
    if key not in _PROGRAM_CACHE:
        _PROGRAM_CACHE[key] = build_program(*key)
    return _PROGRAM_CACHE[key]


# ---------------------------------------------------------------- entry
def _run(inputs, trace=False):
    from concourse.bass_utils import run_bass_kernel_spmd

    x = np.ascontiguousarray(np.asarray(inputs["x"], np.float32))
    assert x.shape == (B, L)
    H0T, H1T = _build_tap_matrices(inputs)
    NC1 = H1T.shape[1]
    I = np.eye(NBLK, dtype=np.float32)

    nc = _get_program(ROWS_PER_CORE, L, NC1)
    xs = x.reshape(N_CORES, ROWS_PER_CORE, L)
    in_maps = [
        {"xin": xs[c], "h0t": H0T, "h1t": H1T, "ident": I}
        for c in range(N_CORES)
    ]
    res = run_bass_kernel_spmd(nc, in_maps, core_ids=list(range(N_CORES)),
                               trace=trace)
    y = np.concatenate(
        [np.asarray(res.results[c]["yout"], np.float32) for c in range(N_CORES)],
        axis=0,
    ).reshape(B, L)
    return y, res


def kernel(x, log_radius, raw_angle, b0, b1, b2):
    y, _ = _run(dict(x=x, log_radius=log_radius, raw_angle=raw_angle,
                     b0=b0, b1=b1, b2=b2))
    return y
